# revision 1
# baseline (speedup 1.0000x reference)
"""Corr1d-x-group cost-volume kernel for Trainium2 (8 NeuronCores, SPMD).

Data-parallel over batch N=8: core i processes batch i.

Per core (inputs [16, 256, 512] f32 each, output [108, 256, 512] f32):
  out[g*27+ch, h, w] = 0.25 * sum_c f1[g*4+c, h, w] * f2[g*4+c, h, w+ch-23]
with zero padding outside w in [0, 512).

Implementation:
  - Inputs are DMA-cast f32->f16 on load (SWDGE cast DMA).
  - SBUF layout per 8-row h-block: partitions = (channel(16), h8(8)) = 128.
  - 27 shifted products on VectorE (fp16 tensor_tensor, 2x perf mode;
    dual parity copies of the padded f2 tile keep slices 4B-aligned).
  - Channel reduction (sum over c of each group g) via TensorE matmul with a
    constant block-diagonal 0.25 weight matrix [128, 32]; 4 shifts packed
    into one PSUM bank via tile_position column groups -> fp32 PSUM.
  - ScalarE copies PSUM->SBUF, HWDGE DMA stores to DRAM.
"""

import os
import numpy as np

import concourse.bass as bass
import concourse.bacc as bacc
import concourse.mybir as mybir
import concourse.tile as tile
from concourse import bass_utils

N, C, H, W = 8, 16, 256, 512
G = 4
TOP_CH = 27
RADIUS = 13
PAD_SHIFT = -10  # shift s = ch - 23 for ch in [0, 27)
OUT_CH = G * TOP_CH  # 108
HB = 32  # h rows per block; 4 channels * 32 rows = 128 partitions
NBLK = H // HB
PADL = 24  # f2 padded tile: column = w + PADL (even so slices align)
F2W = PADL + W + 8  # 544 columns, covers w in [-24, 520)

_CACHED = {}


def _reduction_weights() -> np.ndarray:
    # lhsT [K=(c, h32)=128, M=h32=32]: sums the 4 channels of a group and
    # applies the 1/sumelems scale.
    w = np.zeros((128, 32), np.float16)
    for c in range(G):
        for hh in range(HB):
            w[c * HB + hh, hh] = 0.25
    return w


def _build_program() -> bass.Bass:
    # Bacc (not raw Bass): its compile() splits multi-sem sync waits, which
    # TRN2 hardware limits to one per instruction.
    nc = bacc.Bacc(
        "TRN2",
        target_bir_lowering=False,
        debug=False,
        enable_asserts=False,
        num_devices=N,
    )
    f16 = mybir.dt.float16
    f32 = mybir.dt.float32

    l_in = nc.dram_tensor("l_in", [C, H, W], f32, kind="ExternalInput")
    r_in = nc.dram_tensor("r_in", [C, H, W], f32, kind="ExternalInput")
    w_red = nc.dram_tensor("w_red", [128, 32], f16, kind="ExternalInput")
    out = nc.dram_tensor("out", [OUT_CH, H, W], f32, kind="ExternalOutput")

    # Output viewed as [ch(27), g(4), h*w]: one shift's store for an h-block
    # is [1, 4, HB*W] -> a 2-dim AP against the [128, 512] SBUF stage tile
    # whose partition-major order is (g, h32, w).
    out_v = out.ap().rearrange("(g c) h w -> c g (h w)", g=G)

    with tile.TileContext(nc) as tc:
        with (
            tc.tile_pool(name="wpool", bufs=1) as wpool,
            tc.tile_pool(name="inpool", bufs=2) as inpool,
            tc.tile_pool(name="prodpool", bufs=4) as prodpool,
            tc.tile_pool(name="obpool", bufs=3) as obpool,
            tc.tile_pool(name="psumpool", bufs=2, space="PSUM") as psumpool,
        ):
            wt = wpool.tile([128, 32], f16)
            nc.sync.dma_start(wt[:], w_red[:])

            for ib in range(NBLK):
                h0 = ib * HB
                f1s = []
                f2es = []
                f2os = []
                for g in range(G):
                    f1 = inpool.tile([128, W], f16, tag=f"f1_{g}")
                    nc.gpsimd.dma_start(
                        f1[:], l_in[g * G : (g + 1) * G, h0 : h0 + HB, :]
                    )
                    f1s.append(f1)

                    f2e = inpool.tile([128, F2W], f16, tag=f"f2e_{g}")
                    nc.vector.memset(f2e[:, 0:PADL], 0.0)
                    nc.vector.memset(f2e[:, PADL + W : F2W], 0.0)
                    nc.gpsimd.dma_start(
                        f2e[:, PADL : PADL + W],
                        r_in[g * G : (g + 1) * G, h0 : h0 + HB, :],
                    )
                    f2es.append(f2e)
                    # Odd-parity tile: same data at column = w + (PADL-1), so
                    # odd shifts read from a 4B-aligned start. Loaded with its
                    # own cast-DMA (a DVE shift-copy trips the sync-wait cap).
                    f2o = inpool.tile([128, F2W], f16, tag=f"f2o_{g}")
                    nc.vector.memset(f2o[:, 0 : PADL - 1], 0.0)
                    nc.vector.memset(f2o[:, PADL - 1 + W : F2W], 0.0)
                    nc.gpsimd.dma_start(
                        f2o[:, PADL - 1 : PADL - 1 + W],
                        r_in[g * G : (g + 1) * G, h0 : h0 + HB, :],
                    )
                    f2os.append(f2o)

                for ch in range(TOP_CH):
                    col = PADL + ch - (RADIUS - PAD_SHIFT)  # PADL + shift
                    psumt = psumpool.tile([128, W], f32, tag="psumt")
                    for g in range(G):
                        if col % 2 == 0:
                            src = f2es[g][:, col : col + W]
                        else:
                            src = f2os[g][:, col - 1 : col - 1 + W]
                        p = prodpool.tile([128, W], f16, tag="prod")
                        nc.vector.tensor_mul(p[:], f1s[g][:], src)
                        nc.tensor.matmul(
                            psumt[32 * g : 32 * (g + 1), :],
                            wt[:],
                            p[:],
                            start=True,
                            stop=True,
                            tile_position=(0, 32 * g),
                        )
                    ob = obpool.tile([128, W], f32, tag="ob")
                    nc.scalar.copy(ob[:], psumt[:])
                    nc.sync.dma_start(
                        out_v[ch : ch + 1, :, h0 * W : (h0 + HB) * W],
                        ob[:],
                    )
    nc.compile()
    return nc


def kernel(l_in: np.ndarray, r_in: np.ndarray) -> np.ndarray:
    assert l_in.shape == (N, C, H, W) and r_in.shape == (N, C, H, W)
    l_in = np.ascontiguousarray(l_in, dtype=np.float32)
    r_in = np.ascontiguousarray(r_in, dtype=np.float32)

    if "nc" not in _CACHED:
        _CACHED["nc"] = _build_program()
    nc = _CACHED["nc"]

    w_np = _reduction_weights()
    in_maps = [
        {
            "l_in": np.ascontiguousarray(l_in[i]),
            "r_in": np.ascontiguousarray(r_in[i]),
            "w_red": w_np,
        }
        for i in range(N)
    ]
    trace = bool(int(os.environ.get("CORR_KERNEL_TRACE", "0")))
    kwargs = {}
    tdir = os.environ.get("CORR_KERNEL_TRACE_DIR")
    if trace and tdir:
        os.makedirs(tdir, exist_ok=True)
        kwargs["tmpdir"] = tdir
    res = bass_utils.run_bass_kernel_spmd(
        nc, in_maps, core_ids=list(range(N)), trace=trace, **kwargs
    )
    _CACHED["last_result"] = res
    return np.stack([res.results[i]["out"] for i in range(N)], axis=0)



# revision 4
# speedup vs baseline: 2.4625x; 2.4625x over previous
"""Corr1d-x-group cost-volume kernel for Trainium2 (8 NeuronCores, SPMD).

Data-parallel over batch N=8: core i processes batch i.

Per core (inputs [16, 256, 512], output [108, 256, 512]):
  out[g*27+ch, h, w] = 0.25 * sum_c f1[g*4+c, h, w] * f2[g*4+c, h, w+ch-23]
with zero padding outside w in [0, 512).

v2 design (bottleneck: the 27*8 = 216 shift-product streams on VectorE,
which cap at DVE 2x_1P mode ~= (58 + FD/2)/0.96 ns per op):
  - Host pre-casts inputs to f16 and PRE-ARRANGES them into the exact
    per-block SBUF images (including zero pad columns and the dual parity
    copies of f2), so every device DMA is a contiguous <=3-dim transfer.
    Output is stored f16 in block layout; host un-permutes and upcasts.
  - SBUF layout per 32-row h-block: partitions = (c4, h32) = 128; free dim
    packs all 4 groups, so ONE tensor_mul [128, 4x512] per displacement.
  - Dual parity copies of the padded f2 block keep every shifted slice
    4B-aligned (keeps DVE 2x_1P engaged).
  - Channel reduction via TensorE: 4 col-tiled matmuls (tile_position) with
    a constant [128, 32] block-diagonal 0.25 weight -> fp32 PSUM. 4 shifts
    share one 4-bank PSUM tile.
  - ScalarE evacuates PSUM -> f16 SBUF staging (cast in the ACTIVATE), one
    op per 4 shifts; one HWDGE store per block from the Activation ring,
    loads ride the SP ring.
"""

import os
import numpy as np

import concourse.bass as bass
import concourse.bacc as bacc
import concourse.mybir as mybir
import concourse.tile as tile
from concourse import bass_utils

N, C, H, W = 8, 16, 256, 512
G = 4
TOP_CH = 27
RADIUS = 13
PAD_SHIFT = -10  # shift s = ch - 23 for ch in [0, 27)
OUT_CH = G * TOP_CH  # 108
HB = 32  # h rows per block; 4 channels * 32 rows = 128 partitions
NBLK = H // HB
PADL = 24  # f2 even tile: column = w + PADL within each 544-col group chunk
F2W = PADL + W + 8  # 544 columns per group chunk
PSH = 4  # shifts per PSUM tile (4 banks); last tile holds 3

_CACHED = {}


def _reduction_weights() -> np.ndarray:
    # lhsT [K=(c, h32)=128, M=h32=32]: sums the 4 channels of a group and
    # applies the 1/sumelems scale.
    w = np.zeros((128, 32), np.float16)
    for c in range(G):
        for hh in range(HB):
            w[c * HB + hh, hh] = 0.25
    return w


def _build_program() -> bass.Bass:
    # Bacc (not raw Bass): its compile() splits multi-sem sync waits, which
    # TRN2 hardware limits to one per instruction.
    nc = bacc.Bacc(
        "TRN2",
        target_bir_lowering=False,
        debug=False,
        enable_asserts=False,
        num_devices=N,
    )
    f16 = mybir.dt.float16
    f32 = mybir.dt.float32

    # Pre-arranged block images (see _prep_inputs):
    #   l_blk[ib][c*32+h][g*512+w]      = l[g*4+c, ib*32+h, w]
    #   r_e [ib][c*32+h][g*544+24+w]    = r[g*4+c, ib*32+h, w], pads zero
    #   r_o [ib][c*32+h][g*544+23+w]    = r[g*4+c, ib*32+h, w], pads zero
    l_blk = nc.dram_tensor("l_blk", [NBLK, 128, G * W], f16, kind="ExternalInput")
    r_e = nc.dram_tensor("r_e", [NBLK, 128, G * F2W], f16, kind="ExternalInput")
    r_o = nc.dram_tensor("r_o", [NBLK, 128, G * F2W], f16, kind="ExternalInput")
    w_red = nc.dram_tensor("w_red", [128, 32], f16, kind="ExternalInput")
    # Block-layout output: out_blk[ib][g*32+h][ch*512+w] = out[g*27+ch, ib*32+h, w]
    out = nc.dram_tensor("out", [NBLK, 128, TOP_CH * W], f16, kind="ExternalOutput")

    # PSUM tile chunking: 6 tiles of 4 shifts + 1 tile of 3.
    ps_chunks = [PSH] * 6 + [TOP_CH - PSH * 6]

    with tile.TileContext(nc) as tc:
        with (
            tc.tile_pool(name="wpool", bufs=1) as wpool,
            tc.tile_pool(name="inpool", bufs=2) as inpool,
            tc.tile_pool(name="prodpool", bufs=8) as prodpool,
            tc.tile_pool(name="obpool", bufs=2) as obpool,
            tc.tile_pool(name="psumpool", bufs=2, space="PSUM") as psumpool,
        ):
            wt = wpool.tile([128, 32], f16)
            nc.sync.dma_start(wt[:], w_red[:])

            for ib in range(NBLK):
                f1t = inpool.tile([128, G * W], f16, tag="f1t")
                nc.sync.dma_start(f1t[:], l_blk.ap()[ib : ib + 1])
                f1t3 = f1t[:].rearrange("p (g w) -> p g w", g=G)

                f2e = inpool.tile([128, G * F2W], f16, tag="f2e")
                nc.sync.dma_start(f2e[:], r_e.ap()[ib : ib + 1])
                f2e3 = f2e[:].rearrange("p (g w) -> p g w", g=G)

                f2o = inpool.tile([128, G * F2W], f16, tag="f2o")
                nc.sync.dma_start(f2o[:], r_o.ap()[ib : ib + 1])
                f2o3 = f2o[:].rearrange("p (g w) -> p g w", g=G)

                ob = obpool.tile([128, TOP_CH * W], f16, tag="ob")

                ch = 0
                for psh in ps_chunks:
                    psumt = psumpool.tile([128, PSH * W], f32, tag="ps")
                    for j in range(psh):
                        cc = ch + j
                        # slice start = PADL + (cc - 23) in even tile coords;
                        # even start -> f2o (data at PADL-1), odd -> f2e.
                        if cc % 2 == 0:
                            src = f2o3[:, :, cc : cc + W]
                        else:
                            src = f2e3[:, :, cc + 1 : cc + 1 + W]
                        prod = prodpool.tile([128, G * W], f16, tag="prod")
                        prod3 = prod[:].rearrange("p (g w) -> p g w", g=G)
                        nc.vector.tensor_mul(prod3, f1t3, src)
                        for g in range(G):
                            nc.tensor.matmul(
                                psumt[32 * g : 32 * (g + 1), j * W : (j + 1) * W],
                                wt[:],
                                prod[:, g * W : (g + 1) * W],
                                start=True,
                                stop=True,
                                tile_position=(0, 32 * g),
                            )
                    # PSUM f32 -> SBUF f16 (cast inside the ACTIVATE copy).
                    nc.scalar.copy(
                        ob[:, ch * W : (ch + psh) * W], psumt[:, 0 : psh * W]
                    )
                    ch += psh

                nc.scalar.dma_start(out.ap()[ib : ib + 1], ob[:])
    nc.compile()
    return nc


def _prep_inputs(l16: np.ndarray, r16: np.ndarray):
    """Build per-core block images. l16/r16: [C, H, W] f16 for one core."""
    # [16, 256, 512] -> [g4, c4, b8, h32, w512] -> (b, c, h, g, w)
    l5 = l16.reshape(G, G, NBLK, HB, W).transpose(2, 1, 3, 0, 4)
    l_blk = np.ascontiguousarray(l5).reshape(NBLK, 128, G * W)
    r5 = r16.reshape(G, G, NBLK, HB, W).transpose(2, 1, 3, 0, 4)  # b c h g w
    r_e = np.zeros((NBLK, G, HB, G, F2W), np.float16)
    r_e[:, :, :, :, PADL : PADL + W] = r5
    r_o = np.zeros((NBLK, G, HB, G, F2W), np.float16)
    r_o[:, :, :, :, PADL - 1 : PADL - 1 + W] = r5
    return (
        l_blk,
        r_e.reshape(NBLK, 128, G * F2W),
        r_o.reshape(NBLK, 128, G * F2W),
    )


def _unpack_output(o_blk: np.ndarray) -> np.ndarray:
    """[NBLK, 128, 27*512] f16 block layout -> [108, 256, 512] f32."""
    o5 = o_blk.reshape(NBLK, G, HB, TOP_CH, W).transpose(1, 3, 0, 2, 4)
    return np.ascontiguousarray(o5, dtype=np.float32).reshape(OUT_CH, H, W)


def kernel(l_in: np.ndarray, r_in: np.ndarray) -> np.ndarray:
    assert l_in.shape == (N, C, H, W) and r_in.shape == (N, C, H, W)
    l16 = np.asarray(l_in, dtype=np.float16)
    r16 = np.asarray(r_in, dtype=np.float16)

    if "nc" not in _CACHED:
        _CACHED["nc"] = _build_program()
    nc = _CACHED["nc"]

    w_np = _reduction_weights()
    in_maps = []
    for i in range(N):
        l_blk, r_e, r_o = _prep_inputs(l16[i], r16[i])
        in_maps.append({"l_blk": l_blk, "r_e": r_e, "r_o": r_o, "w_red": w_np})

    trace = bool(int(os.environ.get("CORR_KERNEL_TRACE", "0")))
    kwargs = {}
    tdir = os.environ.get("CORR_KERNEL_TRACE_DIR")
    if trace and tdir:
        os.makedirs(tdir, exist_ok=True)
        kwargs["tmpdir"] = tdir
    res = bass_utils.run_bass_kernel_spmd(
        nc, in_maps, core_ids=list(range(N)), trace=trace, **kwargs
    )
    _CACHED["last_result"] = res
    return np.stack([_unpack_output(res.results[i]["out"]) for i in range(N)], axis=0)


# revision 5
# speedup vs baseline: 2.6121x; 1.0608x over previous
"""Corr1d-x-group cost-volume kernel for Trainium2 (8 NeuronCores, SPMD).

Data-parallel over batch N=8: core i processes batch i.

Per core (inputs [16, 256, 512], output [108, 256, 512]):
  out[g*27+ch, h, w] = 0.25 * sum_c f1[g*4+c, h, w] * f2[g*4+c, h, w+ch-23]
with zero padding outside w in [0, 512).

v3 design (bottleneck: the shift-product stream on VectorE, capped at DVE
2x_1P mode ~= (58 + FD/2)/0.96 ns per op):
  - Host pre-casts inputs to f16 and PRE-ARRANGES them into per-block SBUF
    images (zero pads + dual parity copies of f2 baked in), so every load
    is a contiguous DMA. Output stored f16; host un-permutes and upcasts.
  - SBUF layout per 32-row h-block: partitions = (c4, h32) = 128.
  - DVE work fused into 4 chunk ops per block: each computes 6-7 same-parity
    shifts x 4 groups in ONE tensor_tensor (overlapping strided view of the
    padded f2 tile; f1 broadcast along the shift dim with stride 0).
    Dual parity copies keep every row start 4B-aligned for 2x_1P mode.
  - Channel reduction via TensorE: 4 col-tiled matmuls per shift with a
    constant [128, 32] block-diagonal 0.25 weight -> fp32 PSUM; 4 shifts
    share one 4-bank PSUM tile.
  - ScalarE evacuates PSUM -> f16 SBUF staging (cast in ACTIVATE). Staging
    is in shift-production order (evens then odds); two strided stores per
    block de-interleave channels back to DRAM order.
"""

import os
import numpy as np

import concourse.bass as bass
import concourse.bacc as bacc
import concourse.mybir as mybir
import concourse.tile as tile
from concourse import bass_utils

N, C, H, W = 8, 16, 256, 512
G = 4
TOP_CH = 27
OUT_CH = G * TOP_CH  # 108
HB = 32  # h rows per block; 4 channels * 32 rows = 128 partitions
NBLK = H // HB
PADL = 24  # f2 even tile: column = w + PADL within each 544-col group chunk
F2W = PADL + W + 8  # 544 columns per group chunk
PSH = 4  # shifts per PSUM tile (4 banks); last tile holds 3

# Shift production order: even channels (from the odd-parity tile, aligned
# starts 0,2,..,26) then odd channels (from the even-parity tile, starts
# 2,4,..,26). PROD_CH[k] = output channel of the k-th produced shift.
PROD_CH = list(range(0, TOP_CH, 2)) + list(range(1, TOP_CH, 2))  # 14 + 13
# DVE chunks: (source, start_col, n_shifts); source 'o' = f2o, 'e' = f2e.
CHUNKS = [("o", 0, 7), ("o", 14, 7), ("e", 2, 7), ("e", 16, 6)]

_CACHED = {}


def _reduction_weights() -> np.ndarray:
    # lhsT [K=(c, h32)=128, M=h32=32]: sums the 4 channels of a group and
    # applies the 1/sumelems scale.
    w = np.zeros((128, 32), np.float16)
    for c in range(G):
        for hh in range(HB):
            w[c * HB + hh, hh] = 0.25
    return w


def _ap(base, offset_elems, dims):
    """Raw AP on base's tensor: dims = [[stride, count], ...] in elements."""
    return bass.AP(tensor=base.tensor, offset=base.offset + offset_elems, ap=dims)


def _build_program() -> bass.Bass:
    # Bacc (not raw Bass): its compile() splits multi-sem sync waits, which
    # TRN2 hardware limits to one per instruction.
    nc = bacc.Bacc(
        "TRN2",
        target_bir_lowering=False,
        debug=False,
        enable_asserts=False,
        num_devices=N,
    )
    f16 = mybir.dt.float16
    f32 = mybir.dt.float32

    l_blk = nc.dram_tensor("l_blk", [NBLK, 128, G * W], f16, kind="ExternalInput")
    r_e = nc.dram_tensor("r_e", [NBLK, 128, G * F2W], f16, kind="ExternalInput")
    r_o = nc.dram_tensor("r_o", [NBLK, 128, G * F2W], f16, kind="ExternalInput")
    w_red = nc.dram_tensor("w_red", [128, 32], f16, kind="ExternalInput")
    # out[ib][g*32+h][ch*512+w] = out[g*27+ch, ib*32+h, w] (channel order).
    out = nc.dram_tensor("out", [NBLK, 128, TOP_CH * W], f16, kind="ExternalOutput")
    OBW = TOP_CH * W  # 13824 staging columns per partition

    with tile.TileContext(nc) as tc:
        with (
            tc.tile_pool(name="wpool", bufs=1) as wpool,
            tc.tile_pool(name="inpool", bufs=2) as inpool,
            tc.tile_pool(name="prodpool", bufs=3) as prodpool,
            tc.tile_pool(name="obpool", bufs=2) as obpool,
            tc.tile_pool(name="psumpool", bufs=2, space="PSUM") as psumpool,
        ):
            wt = wpool.tile([128, 32], f16)
            nc.sync.dma_start(wt[:], w_red[:])

            for ib in range(NBLK):
                f1t = inpool.tile([128, G * W], f16, tag="f1t")
                nc.sync.dma_start(f1t[:], l_blk.ap()[ib : ib + 1])

                f2e = inpool.tile([128, G * F2W], f16, tag="f2e")
                nc.sync.dma_start(f2e[:], r_e.ap()[ib : ib + 1])
                f2o = inpool.tile([128, G * F2W], f16, tag="f2o")
                nc.sync.dma_start(f2o[:], r_o.ap()[ib : ib + 1])

                ob = obpool.tile([128, OBW], f16, tag="ob")

                k = 0  # produced-shift index
                psumt = None
                for src_name, col0, m in CHUNKS:
                    src_t = f2o if src_name == "o" else f2e
                    # in1: [p, shift(stride 2), g(stride 544), w(stride 1)]
                    src4 = _ap(
                        src_t[:],
                        col0,
                        [[G * F2W, 128], [2, m], [F2W, G], [1, W]],
                    )
                    prod = prodpool.tile([128, 7 * G * W], f16, tag="prod")
                    prod4 = _ap(
                        prod[:], 0, [[7 * G * W, 128], [G * W, m], [W, G], [1, W]]
                    )
                    f1b = (
                        f1t[:]
                        .rearrange("p (s g w) -> p s g w", s=1, g=G)
                        .broadcast_to([128, m, G, W])
                    )
                    nc.vector.tensor_mul(prod4, f1b, src4)

                    for kl in range(m):
                        j = k % PSH
                        if j == 0:
                            psumt = psumpool.tile([128, PSH * W], f32, tag="ps")
                        for g in range(G):
                            nc.tensor.matmul(
                                psumt[32 * g : 32 * (g + 1), j * W : (j + 1) * W],
                                wt[:],
                                prod[:, (kl * G + g) * W : (kl * G + g + 1) * W],
                                start=True,
                                stop=True,
                                tile_position=(0, 32 * g),
                            )
                        if j == PSH - 1 or k == TOP_CH - 1:
                            # PSUM f32 -> f16 staging (cast in the ACTIVATE).
                            nc.scalar.copy(
                                ob[:, (k - j) * W : (k + 1) * W],
                                psumt[:, 0 : (j + 1) * W],
                            )
                        k += 1

                # De-interleave staging (evens then odds) back to channel
                # order with two strided stores.
                dst_even = _ap(
                    out.ap(), ib * 128 * OBW, [[OBW, 128], [2 * W, 14], [1, W]]
                )
                nc.scalar.dma_start(dst_even, ob[:, 0 : 14 * W])
                dst_odd = _ap(
                    out.ap(), ib * 128 * OBW + W, [[OBW, 128], [2 * W, 13], [1, W]]
                )
                nc.scalar.dma_start(dst_odd, ob[:, 14 * W : OBW])
    nc.compile()
    return nc


def _prep_inputs(l16: np.ndarray, r16: np.ndarray):
    """Build per-core block images. l16/r16: [C, H, W] f16 for one core."""
    # [16, 256, 512] -> [g4, c4, b8, h32, w512] -> (b, c, h, g, w)
    l5 = l16.reshape(G, G, NBLK, HB, W).transpose(2, 1, 3, 0, 4)
    l_blk = np.ascontiguousarray(l5).reshape(NBLK, 128, G * W)
    r5 = r16.reshape(G, G, NBLK, HB, W).transpose(2, 1, 3, 0, 4)  # b c h g w
    r_e = np.zeros((NBLK, G, HB, G, F2W), np.float16)
    r_e[:, :, :, :, PADL : PADL + W] = r5
    r_o = np.zeros((NBLK, G, HB, G, F2W), np.float16)
    r_o[:, :, :, :, PADL - 1 : PADL - 1 + W] = r5
    return (
        l_blk,
        r_e.reshape(NBLK, 128, G * F2W),
        r_o.reshape(NBLK, 128, G * F2W),
    )


def _unpack_output(o_blk: np.ndarray) -> np.ndarray:
    """[NBLK, 128, 27*512] f16 block layout -> [108, 256, 512] f32."""
    o5 = o_blk.reshape(NBLK, G, HB, TOP_CH, W).transpose(1, 3, 0, 2, 4)
    return np.ascontiguousarray(o5, dtype=np.float32).reshape(OUT_CH, H, W)


def kernel(l_in: np.ndarray, r_in: np.ndarray) -> np.ndarray:
    assert l_in.shape == (N, C, H, W) and r_in.shape == (N, C, H, W)
    l16 = np.asarray(l_in, dtype=np.float16)
    r16 = np.asarray(r_in, dtype=np.float16)

    if "nc" not in _CACHED:
        _CACHED["nc"] = _build_program()
    nc = _CACHED["nc"]

    w_np = _reduction_weights()
    in_maps = []
    for i in range(N):
        l_blk, re_, ro_ = _prep_inputs(l16[i], r16[i])
        in_maps.append({"l_blk": l_blk, "r_e": re_, "r_o": ro_, "w_red": w_np})

    trace = bool(int(os.environ.get("CORR_KERNEL_TRACE", "0")))
    kwargs = {}
    tdir = os.environ.get("CORR_KERNEL_TRACE_DIR")
    if trace and tdir:
        os.makedirs(tdir, exist_ok=True)
        kwargs["tmpdir"] = tdir
    res = bass_utils.run_bass_kernel_spmd(
        nc, in_maps, core_ids=list(range(N)), trace=trace, **kwargs
    )
    _CACHED["last_result"] = res
    return np.stack([_unpack_output(res.results[i]["out"]) for i in range(N)], axis=0)
